# revision 111
# baseline (speedup 1.0000x reference)
"""Sparse multi-head attention (per-head strided K/V subsampling) for trn2.

Problem (hardcoded):
  query/key/value: (2048, 8, 512) f32, attn_mask: (8, 2048) bool,
  proj_w: (512, 512), proj_b: (512,).
  Per head h (8 heads, head_dim 64) with stride ksz in [4,4,2,2,1,1,1,1]:
    scores = q_h @ k_h[::ksz].T * 0.125, masked softmax over subsampled keys,
    o_h = softmax @ v_h[::ksz].
  Reference then does a RAW reshape (B,T,D)->(T,B,D) per head before concat +
  out-projection.  That reshape is a pure row permutation of the flattened
  (B*T, 512) matrix, so computing per-(batch,head) attention in (t, d) layout,
  concatenating per batch, projecting, stacking batches, and reshaping
  (B*T, 512) -> (T, B, 512) reproduces it exactly.

Sharding: batch-parallel, one batch element per NeuronCore (8 cores).

Device/layout design (measured-on-HW rationale; ~149us vs 165us for the
earlier no-tails/dup-q variant):
  - mask-gather on the host: masked keys contribute exactly zero, so only
    unmasked subsampled keys are shipped (~50%). Pad rows are all-zero
    INCLUDING the ones-column of the V-augmented matrix, so pads add 0 to
    both numerator and denominator (their exp(0)=1 hits zero V rows).
  - all matmul operands fp16 (f32r streams ~3x slower per row on real HW;
    fp8 fails the max-norm gate: per-term e4m3 error does NOT average down
    in a dot product, rel-to-absmax would be ~3-5e-2).
  - scores computed transposed (s on partitions, t free); V augmented with a
    ones column so one accumulating matmul produces both the unnormalized
    output (rows 0:64 of po) and the softmax denominator (row 64).
  - the two score matmuls of a chunk run on opposite PE row groups (row
    tiling) via 64-row-swapped operand copies; since the natural copy is
    only read at even 512-col t-blocks and the swapped at odd ones, q ships
    as ONE column-interleaved [E, T] tensor and k as [P, 2N] (A|B in
    columns) - DMA here is per-partition-packet-bound, so fewer/wider rows
    is what matters, not bytes.
  - the heads of a pair share their keep-set, so their <=64-lane ragged
    tails are PACKED into one shared chunk (head-even keys at partitions
    0:64, head-odd at 64:128): one ACTIVATE instead of two per pair per
    t-half (96 total instead of 104; ScalarE/ACT at (N+352)/1.2ns per inst
    is the pacing engine, ~107us busy). The tail rides at the end of the
    even head's chunk stream; its 4 PVs close po0 and open po1 and are
    emitted interleaved so they pair on PE row groups.
  - carry-based software pipeline: each head's final PV (+norm) is emitted
    only after the NEXT head's first scores+exp, so the lag-1 pipeline
    never drains at head/pair boundaries (the in-order Tensor queue would
    otherwise head-block ~1.3us per boundary).
  - normalization: den copy to a partition-0 tile (custom DVE ops misread
    non-zero base partitions; PSUM reads must be partition-aligned), oU
    copy releases po, then reciprocal_approx_fast + gpsimd
    partition-broadcast + DVE multiply off the critical path. The very
    last norm runs a split-column fast path (two 512-col half-chains,
    multiplies straight from PSUM, den copies on the now-idle ScalarE -
    the tail is DVE-queue-bound) so the projection tail starts earlier.
  - out-projection with host-side proj_w.T; bias added via DVE tensor_add
    during the PSUM->SBUF copy. 6 of 16 projection chunks are injected
    into t-half 1 (odd heads at j=3 borrow the po slot the even head's
    norm released; others borrow a ps slot); the 10 tail chunks run as 5
    PAIRS sharing one [P,THALF] accumulator each (cols 0:512/512:1024,
    separate region-tracked adds so chunk A's add+DMA overlap chunk B's
    MMs), alternating po/ps slots (PSUM is free then) so the MM stream
    never waits on an add to release an accumulator. A single wide
    [128,1024] add measured WORSE (serializes the pair). The tile scheduler hoists injected
    chunks to the moment t-half-0's last norm lands regardless of emission
    slot - model-time floors (tile_wait_until) made this worse, not
    better, and so did explicit pair-3 injection slots for the two
    hoisted th0-column chunks.
  - dummy matmul bursts keep the PE HAM clock gate warm: 10x N=512 during
    the initial DMA wait, 8x N=256 while the final norm chain runs (the
    HAM ramps ~14us after first sustained activity; cold-clock MMs run at
    ~1.4GHz vs 2.4).
"""

import numpy as np

import concourse.bass as bass
import concourse.tile as tile
from concourse import bacc, mybir
from concourse.bass_utils import run_bass_kernel_spmd

T = 2048
B = 8
E = 512
H = 8
D = 64
KS = [4, 4, 2, 2, 1, 1, 1, 1]
SCALE = 0.125
P = 128
THALF = 1024
F32 = mybir.dt.float32
F16 = mybir.dt.float16


def build_program(nf4, nf2, nf1, t4, t2, t1, bias0):
    # nfX: number of FULL 128-lane chunks per stride group; tX: the group
    # has a <=64-lane ragged tail, which is packed PAIRWISE (head-even keys
    # on partitions 0:64, head-odd on 64:128) into one shared chunk - one
    # ACTIVATE instead of two per pair per t-half. Pad lanes contribute
    # exactly zero to numerator and denominator.
    N4, N2, N1 = nf4 * P, nf2 * P, nf1 * P
    nc = bacc.Bacc("TRN2", target_bir_lowering=False, debug=False, num_devices=B)

    # k tensors hold the natural (A) and half-swapped (B) copies side by
    # side in columns: [P, 2N] makes one wide DMA (128 packets) instead of
    # two thin ones (256 packets) - DMA cost here is per-partition-packet.
    # q is stored COLUMN-INTERLEAVED: even 512-col blocks hold the natural
    # (A) copy, odd blocks the 64-row-swapped (B) copy - each tq score MM
    # only ever reads its own parity's columns, so one [E, T] tensor (and
    # one 2KB-row DMA per group/t-half) replaces the 2E duplicated layout
    qT = nc.dram_tensor("qT", [E, T], F16, kind="ExternalInput")
    k4T = nc.dram_tensor("k4T", [P, 2 * N4], F16, kind="ExternalInput")
    k2T = nc.dram_tensor("k2T", [P, 2 * N2], F16, kind="ExternalInput")
    k1Ta = nc.dram_tensor("k1Ta", [P, 2 * N1], F16, kind="ExternalInput")
    k1Tb = nc.dram_tensor("k1Tb", [P, 2 * N1], F16, kind="ExternalInput")
    va4 = nc.dram_tensor("va4", [P, nf4 * 130], F16, kind="ExternalInput")
    va2 = nc.dram_tensor("va2", [P, nf2 * 130], F16, kind="ExternalInput")
    va1 = nc.dram_tensor("va1", [P, nf1 * 260], F16, kind="ExternalInput")
    kt4 = nc.dram_tensor("kt4", [P, 128], F16, kind="ExternalInput")
    kt2 = nc.dram_tensor("kt2", [P, 128], F16, kind="ExternalInput")
    kt1 = nc.dram_tensor("kt1", [P, 256], F16, kind="ExternalInput")
    vat4 = nc.dram_tensor("vat4", [P, 130], F16, kind="ExternalInput")
    vat2 = nc.dram_tensor("vat2", [P, 130], F16, kind="ExternalInput")
    vat1 = nc.dram_tensor("vat1", [P, 260], F16, kind="ExternalInput")
    wT = nc.dram_tensor("wT", [E, E], F16, kind="ExternalInput")
    pb = nc.dram_tensor("pb", [1, E], F32, kind="ExternalInput")
    out = nc.dram_tensor("out", [T, E], F32, kind="ExternalOutput")

    NCHF = [nf4, nf4, nf2, nf2, nf1, nf1, nf1, nf1]
    TAIL = [t4, t4, t2, t2, t1, t1, t1, t1]

    with tile.TileContext(nc) as tc:
        with (
            tc.tile_pool(name="const", bufs=1) as cpool,
            tc.tile_pool(name="exp", bufs=4) as epool,
            tc.tile_pool(name="norm", bufs=3) as npool,
            tc.tile_pool(name="outsb", bufs=4) as opool,
            tc.tile_pool(name="psA", bufs=1, space="PSUM") as pspool,
        ):
            # ---- persistent SBUF loads (ordered by first use) ----
            # q tiles are DMAed in t-half column slices: the t-half-1
            # columns aren't touched until ~60% into the kernel, so
            # deferring them halves the q bytes on the startup critical
            # path (first scores MM needs qW0 cols 0:THALF + k4).
            qW_sb = []
            for p_ in range(4):
                qW_sb.append(cpool.tile([P, T], F16, name=f"qW{p_}",
                                        tag=f"qW{p_}"))

            def qdma(p_, th, eng=None):
                c0 = th * THALF
                (eng or nc.sync).dma_start(
                    qW_sb[p_][:, c0:c0 + THALF],
                    qT.ap()[p_ * P:(p_ + 1) * P, c0:c0 + THALF])

            def kload(name, dram, N, eng):
                t_ = cpool.tile([P, 2 * N], F16, name=name, tag=name)
                eng.dma_start(t_[:], dram.ap())
                return t_

            # the scalar engine is the second HWDGE (~96GB/s vs ~196GB/s on
            # sync): k/va loads for pairs 0-1 stream there concurrently with
            # the q tiles on the sync queue, pulling the first scores MM
            # several us earlier; its issue instructions all run during the
            # pre-compute window so they never delay the exp ACTIVATEs
            def tdma(name, dram, W, eng):
                t_ = cpool.tile([P, W], F16, name=name, tag=name)
                eng.dma_start(t_[:], dram.ap())
                return t_

            # startup split: DMA latency here is packet-count-bound (one
            # packet per partition row), so the first chunk's deps are
            # split across both HWDGE queues by partition halves
            k4_sb = kload("k4", k4T, N4, nc.sync)
            nc.scalar.dma_start(qW_sb[0][64:P, 0:THALF],
                                qT.ap()[64:P, 0:THALF])
            nc.sync.dma_start(qW_sb[0][0:64, 0:THALF],
                              qT.ap()[0:64, 0:THALF])
            va4_sb = cpool.tile([P, nf4 * 130], F16, name="va4s", tag="va4s")
            nc.sync.dma_start(va4_sb[0:64, :], va4.ap()[0:64, :])
            nc.scalar.dma_start(va4_sb[64:P, :], va4.ap()[64:P, :])
            k2_sb = kload("k2", k2T, N2, nc.scalar)
            va2_sb = cpool.tile([P, nf2 * 130], F16, name="va2s", tag="va2s")
            nc.scalar.dma_start(va2_sb[:], va2.ap())
            kt4_sb = tdma("kt4s", kt4, 128, nc.sync) if t4 else None
            vat4_sb = tdma("vat4s", vat4, 130, nc.sync) if t4 else None
            qdma(1, 0)
            kt2_sb = tdma("kt2s", kt2, 128, nc.scalar) if t2 else None
            vat2_sb = tdma("vat2s", vat2, 130, nc.scalar) if t2 else None
            qdma(2, 0)
            k1a_sb = kload("k1a", k1Ta, N1, nc.sync)
            qdma(3, 0)
            k1b_sb = kload("k1b", k1Tb, N1, nc.sync)
            va1_sb = cpool.tile([P, nf1 * 260], F16, name="va1s", tag="va1s")
            nc.sync.dma_start(va1_sb[:], va1.ap())
            kt1_sb = tdma("kt1s", kt1, 256, nc.sync) if t1 else None
            vat1_sb = tdma("vat1s", vat1, 260, nc.sync) if t1 else None
            for p_ in range(4):
                qdma(p_, 1)
            wT_sb = []
            for i in range(4):
                t_ = cpool.tile([P, E], F16, name=f"wT{i}", tag=f"wT{i}")
                nc.sync.dma_start(t_[:], wT.ap()[i * P:(i + 1) * P, :])
                wT_sb.append(t_)
            pb_sb = cpool.tile([1, E], F32, name="pbs", tag="pbs")
            nc.sync.dma_start(pb_sb[:], pb.ap())
            pbb_sb = cpool.tile([P, E], F32, name="pbb", tag="pbb")
            nc.gpsimd.partition_broadcast(pbb_sb[:], pb_sb[:])

            # ---- PE warm-up burst ----
            wu_sb = cpool.tile([64, 512], F16, name="wu", tag="wu")
            nc.gpsimd.memset(wu_sb[:], 0.0)
            ones_sb = cpool.tile([1, P], F16, name="ones", tag="ones")
            nc.gpsimd.memset(ones_sb[:], 1.0)
            # prime the ACT exp table: the first ACTIVATE of a new function
            # set pays ~2.7us of table load + drain; trigger it on a dummy
            # during the initial DMA wait instead of on the first real exp
            prime_sb = cpool.tile([1, 16], F32, name="prime", tag="prime")
            nc.scalar.activation(prime_sb[:], wu_sb[0:1, 0:16],
                                 mybir.ActivationFunctionType.Exp,
                                 bias=0.0, scale=1.0)

            def warm_burst(n, nfree=512):
                wps = pspool.tile([P, THALF], F32, name="ps", tag="ps",
                                  bufs=2)
                for _ in range(n):
                    nc.tensor.matmul(
                        wps[0:4, 0:nfree], lhsT=wu_sb[:, 0:4],
                        rhs=wu_sb[:, 0:nfree], start=True, stop=True)

            warm_burst(10)

            # per-head views: (tile, row0, col0); the B (half-swapped) copy
            # lives in the right column half of the combined k tile
            def kT_h(h, tq):
                kt, N = [(k4_sb, N4), (k4_sb, N4), (k2_sb, N2), (k2_sb, N2),
                         (k1a_sb, N1), (k1a_sb, N1), (k1b_sb, N1),
                         (k1b_sb, N1)][h]
                if tq == 0:
                    return kt, (h % 2) * 64, 0
                return kt, (1 - h % 2) * 64, N

            def qT_h(h, tq):
                if tq == 0:
                    return qW_sb[h // 2], (h % 2) * 64
                return qW_sb[h // 2], (1 - h % 2) * 64

            def va_h(h, j):
                if h < 2:
                    return va4_sb[:, j * 130 + h * 65: j * 130 + h * 65 + 65]
                if h < 4:
                    return va2_sb[:, j * 130 + (h - 2) * 65:
                                  j * 130 + (h - 2) * 65 + 65]
                return va1_sb[:, j * 260 + (h - 4) * 65:
                              j * 260 + (h - 4) * 65 + 65]

            # transposed normalized head outputs (fp16), feeding proj
            oT_sb = []
            for p_ in range(4):
                t_ = cpool.tile([P, T], F16, name=f"oT{p_}", tag=f"oT{p_}")
                oT_sb.append(t_)

            _pp_live = {}

            def proj_half(tq, phase, tag="ps", eng=None):
                # phase 0: first two K-group MMs; phase 1: last two + bias
                # add + out DMA. Splitting a po-slot injection across two
                # adjacent chunk slots keeps the per-slot Tensor-queue load
                # (~430ns) inside the per-chunk PE slack (only safe for
                # po-tag slots: a ps slot must not be held across a chunk
                # boundary).
                if phase == 0:
                    pp_full = pspool.tile([P, THALF], F32, name="pp",
                                          tag=tag, bufs=2)
                    _pp_live[tq] = pp_full
                else:
                    pp_full = _pp_live.pop(tq)
                pp = pp_full[:, 0:E]
                for i in (0, 1) if phase == 0 else (2, 3):
                    nc.tensor.matmul(
                        pp, lhsT=oT_sb[i][:, tq * P:(tq + 1) * P],
                        rhs=wT_sb[i][:], start=(i == 0), stop=(i == 3))
                if phase == 1:
                    ot = opool.tile([P, E], F32, name="ot", tag="ot")
                    nc.vector.tensor_add(ot[:], pp, pbb_sb[:])
                    (eng or nc.sync).dma_start(
                        out.ap()[tq * P:(tq + 1) * P, :], ot[:])

            def proj_chunk(tq, tag="ps", eng=None):
                proj_half(tq, 0, tag=tag)
                proj_half(tq, 1, tag=tag, eng=eng)

            def proj_pair(tqA, tag="ps"):
                # two adjacent t-blocks share one [P, THALF] accumulator
                # (cols 0:512 / 512:1024). Chunk A: DVE bias-add; chunk B:
                # when the bias is all-zero (selected at runtime like the
                # mask-derived cfg), a pure PSUM->SBUF Copy on the idle
                # ScalarE - the tail's serial add stream then runs on two
                # engines in parallel.
                pp_full = pspool.tile([P, THALF], F32, name="pp", tag=tag,
                                      bufs=2)
                for s, tq in ((0, tqA), (512, tqA + 1)):
                    pp = pp_full[:, s:s + E]
                    for i in range(4):
                        nc.tensor.matmul(
                            pp, lhsT=oT_sb[i][:, tq * P:(tq + 1) * P],
                            rhs=wT_sb[i][:], start=(i == 0), stop=(i == 3))
                ot = opool.tile([P, THALF], F32, name="otw", tag="ot")
                nc.vector.tensor_add(ot[:, 0:E], pp_full[:, 0:E],
                                     pbb_sb[:])
                nc.vector.tensor_add(ot[:, E:THALF],
                                     pp_full[:, E:THALF], pbb_sb[:])
                for s, tq in ((0, tqA), (512, tqA + 1)):
                    nc.sync.dma_start(out.ap()[tq * P:(tq + 1) * P, :],
                                      ot[:, s:s + E])

            def norm(h, po_, t0, fast=False, act_recip=False):
                # standard path: two DVE copies release po quickly (den must
                # be a partition-0 tile: custom DVE ops misread non-zero base
                # partitions; PSUM reads must start at an aligned partition);
                # recip/broadcast/multiply run from SBUF off the fast path.
                # fast path (tail only, po release timing irrelevant):
                # multiply straight from PSUM, skipping the oU copy.
                # act_recip (very last norm only): 1/den = exp(-ln(den)) on
                # the now-idle ScalarE (ln and exp share the
                # natural_log_exp table set - no table switch), broadcast
                # via a K=1 PE matmul into free PSUM instead of the 1.76us
                # gpsimd broadcast: chain ~4.3us instead of ~6.6us.
                r0 = (h % 2) * 64
                if act_recip:
                    # very last norm: process in two 512-column halves so
                    # the first projection chunks (which read the low t
                    # columns) start earlier; both den->recip chains are
                    # issued BEFORE the multiplies so the second gpsimd
                    # broadcast isn't serialized behind the first multiply
                    # on the DVE queue; multiply straight from PSUM (one
                    # PSUM operand is allowed)
                    rbcs = []
                    for c in (0, 512):
                        den = npool.tile([1, 512], F32, name="denh",
                                         tag="den")
                        # ScalarE is idle after the final exp: den copies
                        # there relieve the DVE queue that paces the whole
                        # tail (Copy is in every ACT table set - no reload)
                        nc.scalar.activation(
                            den[:], po_[64:65, c:c + 512],
                            mybir.ActivationFunctionType.Copy,
                            bias=0.0, scale=1.0)
                        rec = npool.tile([1, 512], F32, name="rech",
                                         tag="rec")
                        nc.vector.reciprocal_approx_fast(rec[:], den[:])
                        rbc = npool.tile([64, 512], F32, name="rbch",
                                         tag="rbc")
                        nc.gpsimd.partition_broadcast(rbc[:], rec[:])
                        rbcs.append(rbc)
                    for i, c in enumerate((0, 512)):
                        nc.vector.tensor_mul(
                            oT_sb[h // 2][r0:r0 + 64,
                                          t0 + c:t0 + c + 512],
                            po_[0:64, c:c + 512], rbcs[i][:])
                    return
                den = npool.tile([1, THALF], F32, name="den", tag="den")
                nc.vector.tensor_copy(den[:], po_[64:65, :])
                rec = npool.tile([1, THALF], F32, name="rec", tag="rec")
                if not fast:
                    oU = npool.tile([64, THALF], F32, name="oU", tag="oU")
                    nc.vector.tensor_copy(oU[:], po_[0:64, :])
                    nc.vector.reciprocal_approx_fast(rec[:], den[:])
                    src = oU[:]
                else:
                    nc.vector.reciprocal_approx_fast(rec[:], den[:])
                    src = po_[0:64, :]
                rbc = npool.tile([64, THALF], F32, name="rbc", tag="rbc")
                nc.gpsimd.partition_broadcast(rbc[:], rec[:])
                nc.vector.tensor_mul(
                    oT_sb[h // 2][r0:r0 + 64, t0:t0 + THALF], src, rbc[:])

            def tail_tiles(pr):
                # (kt, vat, c0) for pair pr; c0 = column base within the
                # shared stride-1 tail tensors (pairs (4,5) and (6,7))
                if pr == 0:
                    return kt4_sb, vat4_sb, 0
                if pr == 1:
                    return kt2_sb, vat2_sb, 0
                if pr == 2:
                    return kt1_sb, vat1_sb, 0
                return kt1_sb, vat1_sb, 64

            # ---- attention main loop ----
            # carry[0] holds the previous head's final PV emission: it is
            # issued only after the NEXT head's first scores+exp, so the
            # lag-1 software pipeline never drains at head boundaries (the
            # last PV waiting on the last exp used to stall the next head's
            # scores on the in-order Tensor queue, a ~1.3us ACT bubble per
            # boundary). norms are deferred the same way.
            carry = [None]
            for th in range(2):
                t0 = th * THALF
                for pr in range(4):
                    h0, h1 = 2 * pr, 2 * pr + 1
                    nf = NCHF[h0]
                    hastail = TAIL[h0]
                    po0 = pspool.tile([P, THALF], F32, name="po", tag="po",
                                      bufs=2)
                    po1 = pspool.tile([P, THALF], F32, name="po", tag="po",
                                      bufs=2)

                    def pv(h, po_, ex_, j, last, start=None):
                        for tq in range(2):
                            nc.tensor.matmul(
                                po_[0:65, tq * 512:(tq + 1) * 512],
                                lhsT=va_h(h, j),
                                rhs=ex_[:, tq * 512:(tq + 1) * 512],
                                start=(j == 0) if start is None else start,
                                stop=last)

                    def tail_chunk(t0_, pr_=pr):
                        # shared pair tail: head-even keys at partitions
                        # 0:64 (A columns of kt / left vat block), head-odd
                        # at 64:128; 4 score MMs land on 4 distinct PE
                        # quadrants and run pairwise-concurrently, one exp
                        # covers both heads
                        ktt, vat, c0 = tail_tiles(pr_)
                        W = 128 if pr_ >= 2 else 64
                        ps = pspool.tile([P, THALF], F32, name="ps",
                                         tag="ps", bufs=2)
                        for hh, rbase in ((2 * pr_, 0), (2 * pr_ + 1, 64)):
                            for tq in range(2):
                                kr = (hh % 2) * 64 if tq == 0 \
                                    else (1 - hh % 2) * 64
                                kc = c0 + (0 if tq == 0 else W)
                                qt, qr = qT_h(hh, tq)
                                nc.tensor.matmul(
                                    ps[rbase:rbase + 64,
                                       tq * 512:(tq + 1) * 512],
                                    lhsT=ktt[kr:kr + 64, kc:kc + 64],
                                    rhs=qt[qr:qr + 64, t0_ + tq * 512:
                                           t0_ + (tq + 1) * 512],
                                    start=True, stop=True)
                        ext = epool.tile([P, THALF], F16, name="ex",
                                         tag="ex", bufs=6)
                        nc.scalar.activation(
                            ext[:], ps[:], mybir.ActivationFunctionType.Exp,
                            bias=0.0, scale=SCALE)
                        return ext

                    def tail_pvs(po0_, po1_, ext, vc0, pr_=pr):
                        # interleaved emission: the po0 (rows 0:64) and po1
                        # (rows 64:128) MMs of the same tq pair up on
                        # opposite PE row groups and run concurrently
                        vat = tail_tiles(pr_)[1]
                        for tq in range(2):
                            for po_, rbase, vc, st, sp in (
                                    (po0_, 0, vc0, False, True),
                                    (po1_, 64, vc0 + 65, True, False)):
                                nc.tensor.matmul(
                                    po_[0:65, tq * 512:(tq + 1) * 512],
                                    lhsT=vat[rbase:rbase + 64, vc:vc + 65],
                                    rhs=ext[rbase:rbase + 64,
                                            tq * 512:(tq + 1) * 512],
                                    start=st, stop=sp)

                    def full_head(h, po_, t0_, with_tail=False,
                                  pre_started=False):
                        exs = []
                        for j in range(nf):
                            ps = pspool.tile([P, THALF], F32, name="ps",
                                             tag="ps", bufs=2)
                            for tq in range(2):
                                kt, kr, kc = kT_h(h, tq)
                                qt, qr = qT_h(h, tq)
                                nc.tensor.matmul(
                                    ps[:, tq * 512:(tq + 1) * 512],
                                    lhsT=kt[kr:kr + 64,
                                            kc + j * P:kc + (j + 1) * P],
                                    rhs=qt[qr:qr + 64, t0_ + tq * 512:
                                           t0_ + (tq + 1) * 512],
                                    start=True, stop=True)
                            ex = epool.tile([P, THALF], F16, name="ex",
                                            tag="ex", bufs=6)
                            nc.scalar.activation(
                                ex[:], ps[:],
                                mybir.ActivationFunctionType.Exp,
                                bias=0.0, scale=SCALE)
                            exs.append(ex)
                            # fire the carry at j=1 (not j=0): the
                            # previous norm's DVE reads then have one more
                            # ACT period to release the po slot before the
                            # carried tail-PV (start=True) needs it - that
                            # wait head-blocked the Tensor queue ~770ns at
                            # every pair boundary
                            if j == 1 and carry[0] is not None:
                                carry[0]()
                                carry[0] = None
                            if j >= 1:
                                pv(h, po_, exs[j - 1], j - 1, last=False,
                                   start=(j == 1 and not pre_started))
                            # inject t-half-0 projection chunks into t-half
                            # 1: odd heads borrow the po slot freed by the
                            # even head's norm; the rest borrow a ps slot
                            # in the long heads, spaced >=2 chunks apart so
                            # the ACT backlog can recover
                            if th == 1:
                                po_sched = {(3, 3): 0, (5, 3): 1,
                                            (7, 3): 2}
                                ps_sched = {(2, 2): 3, (4, 5): 4,
                                            (5, 6): 5}
                                if (h, j) in po_sched:
                                    proj_chunk(po_sched[(h, j)], tag="po")
                                elif (h, j) in ps_sched:
                                    proj_chunk(ps_sched[(h, j)])
                        # the pair's shared tail chunk rides at the end of
                        # the EVEN head's chunk stream; pv(nf-1) takes its
                        # natural lag-1 slot during the tail's ACT so the
                        # carry slot only holds the 4 interleaved tail PVs
                        # (close po0, open po1) - keeps the pair-boundary
                        # Tensor-queue load under one ACT period
                        if with_tail:
                            ext = tail_chunk(t0_)
                            pv(h, po_, exs[nf - 1], nf - 1, last=False)

                            def finish(poo=po_, ext=ext, po1_=po1,
                                       vc0=tail_tiles(pr)[2] // 64 * 130):
                                tail_pvs(poo, po1_, ext, vc0)
                        else:
                            def finish(hh=h, poo=po_, ex_=exs[nf - 1],
                                       jj=nf - 1):
                                pv(hh, poo, ex_, jj, last=True)
                        return finish

                    last = th == 1 and pr == 3
                    endth = pr == 3
                    fin0 = full_head(h0, po0, t0, with_tail=hastail)
                    # the even head's last PV + norm are deferred into the
                    # odd head's first chunk slot; the norm DVE chain then
                    # overlaps h1's processing
                    def carry0(fin0=fin0, h0=h0, po0=po0, t0=t0, last=last):
                        fin0()
                        norm(h0, po0, t0, fast=last)
                    carry[0] = carry0
                    fin1 = full_head(h1, po1, t0, pre_started=hastail)
                    if last:
                        fin1()
                        # dummy matmuls issued BEFORE the last norm so the
                        # Tensor queue runs them during the norm chain
                        # (keeps the PE HAM clock warm into the projection
                        # tail); short N so they fill the window densely
                        warm_burst(8, 256)
                        norm(h1, po1, t0, fast=True, act_recip=True)
                    else:
                        def carry1(fin1=fin1, h1=h1, po1=po1, t0=t0):
                            fin1()
                            norm(h1, po1, t0)
                        carry[0] = carry1
            # tail projection: PSUM is free now, so alternate po/ps slots
            # (4 accumulators) - the MM stream never waits on an add to
            # release a slot
            # first tail pair (th0 columns) is hoistable by the
            # scheduler into the last attention pair - give it a po slot
            # (pipeline-safe borrow) rather than a ps slot (which would
            # single-buffer the score stream mid-pair)
            for i, tqA in enumerate(range(6, 16, 2)):
                proj_pair(tqA, tag="ps" if i % 2 else "po")

    nc.compile()
    return nc


_PROGRAMS = {}


def _get_program(key):
    if key not in _PROGRAMS:
        _PROGRAMS[key] = build_program(*key)
    return _PROGRAMS[key]


def _swap_halves(m):
    # [128k, N] -> swap the two 64-row halves within each 128-row block
    blocks = [m[i:i + P] for i in range(0, m.shape[0], P)]
    return np.vstack([np.vstack([b[64:P], b[0:64]]) for b in blocks])


def _prep_core_inputs(query, key, value, wT, pb, keeps, cfg):
    nf4, nf2, nf1 = cfg[0], cfg[1], cfg[2]
    NF = {4: nf4, 2: nf2, 1: nf1}
    ins = []
    for b in range(B):
        qb = np.ascontiguousarray(query[:, b, :].T).astype(np.float16)
        # column-interleave: even 512-col blocks natural, odd blocks with
        # the 64-row halves swapped (see the qT dram tensor comment)
        qs = _swap_halves(qb)
        qbd = qb.copy()
        for c in (512, 1536):
            qbd[:, c:c + 512] = qs[:, c:c + 512]

        def build_k(sub, idx, c0, c1, ks):
            # [P, 2N]: natural copy in the left half, 64-row-swapped copy in
            # the right half (one wide DMA instead of two thin ones)
            z = np.zeros((P, NF[ks] * P), dtype=np.float16)
            g = sub[idx[:NF[ks] * P]]
            n = g.shape[0]
            z[:, 0:n] = g[:, c0:c1].T.astype(np.float16)
            return np.hstack([z, np.vstack([z[64:P], z[0:64]])])

        def build_va(sub, idx, heads, W, ks):
            g = sub[idx[:NF[ks] * P]]
            z = np.zeros((P, NF[ks] * W), dtype=np.float16)
            for j in range(NF[ks]):
                seg = g[j * P:(j + 1) * P]
                m = seg.shape[0]
                if m == 0:
                    break
                for i, h in enumerate(heads):
                    z[0:m, j * W + i * 65: j * W + i * 65 + 64] = \
                        seg[:, h * 64:(h + 1) * 64].astype(np.float16)
                    z[0:m, j * W + i * 65 + 64] = 1.0
            return z

        def build_kt(sub, idx, pairs, ks):
            # pair tail keys as columns: head-even q-dims on rows 0:64,
            # head-odd on 64:128; combined [P, 2W] with the half-swapped
            # copy in the right column block
            W = 64 * len(pairs)
            z = np.zeros((P, W), dtype=np.float16)
            g = sub[idx[NF[ks] * P:]]
            n = g.shape[0]
            for i, (ha, hb) in enumerate(pairs):
                if n:
                    z[0:64, i * 64:i * 64 + n] = \
                        g[:, ha * 64:(ha + 1) * 64].T.astype(np.float16)
                    z[64:P, i * 64:i * 64 + n] = \
                        g[:, hb * 64:(hb + 1) * 64].T.astype(np.float16)
            return np.hstack([z, np.vstack([z[64:P], z[0:64]])])

        def build_vat(sub, idx, pairs, ks):
            # head-even tail keys at partitions 0:n (left 65-col block),
            # head-odd at 64:64+n (right block), ones column last
            z = np.zeros((P, 130 * len(pairs)), dtype=np.float16)
            g = sub[idx[NF[ks] * P:]]
            n = g.shape[0]
            for i, (ha, hb) in enumerate(pairs):
                if n:
                    z[0:n, i * 130:i * 130 + 64] = \
                        g[:, ha * 64:(ha + 1) * 64].astype(np.float16)
                    z[0:n, i * 130 + 64] = 1.0
                    z[64:64 + n, i * 130 + 65:i * 130 + 129] = \
                        g[:, hb * 64:(hb + 1) * 64].astype(np.float16)
                    z[64:64 + n, i * 130 + 129] = 1.0
            return z

        kb, vb = key[:, b, :], value[:, b, :]
        i4, i2, i1 = keeps[4][b], keeps[2][b], keeps[1][b]
        ins.append({
            "qT": qbd,
            "k4T": build_k(kb[::4], i4, 0, 128, 4),
            "k2T": build_k(kb[::2], i2, 128, 256, 2),
            "k1Ta": build_k(kb, i1, 256, 384, 1),
            "k1Tb": build_k(kb, i1, 384, 512, 1),
            "va4": build_va(vb[::4], i4, [0, 1], 130, 4),
            "va2": build_va(vb[::2], i2, [2, 3], 130, 2),
            "va1": build_va(vb, i1, [4, 5, 6, 7], 260, 1),
            "kt4": build_kt(kb[::4], i4, [(0, 1)], 4),
            "kt2": build_kt(kb[::2], i2, [(2, 3)], 2),
            "kt1": build_kt(kb, i1, [(4, 5), (6, 7)], 1),
            "vat4": build_vat(vb[::4], i4, [(0, 1)], 4),
            "vat2": build_vat(vb[::2], i2, [(2, 3)], 2),
            "vat1": build_vat(vb, i1, [(4, 5), (6, 7)], 1),
            "wT": wT, "pb": pb,
        })
    return ins


def kernel(query, key, value, attn_mask, proj_w, proj_b, _trace=False,
           **run_kwargs):
    query = np.asarray(query, dtype=np.float32)
    key = np.asarray(key, dtype=np.float32)
    value = np.asarray(value, dtype=np.float32)
    mask = np.asarray(attn_mask).astype(bool)
    wT = np.ascontiguousarray(
        np.asarray(proj_w, dtype=np.float32).T).astype(np.float16)
    pb = np.ascontiguousarray(
        np.asarray(proj_b, dtype=np.float32).reshape(1, E))

    keeps = {ks: [np.flatnonzero(~mask[b, ::ks]) for b in range(B)]
             for ks in (4, 2, 1)}
    # <=64-lane ragged tails are packed pairwise into one shared chunk
    # (saves one ACTIVATE per pair per t-half); wider tails are padded up
    # to a full 128-lane chunk
    nfs, tails = [], []
    for ks in (4, 2, 1):
        mx = max(len(keeps[ks][b]) for b in range(B))
        nfull, rem = divmod(mx, P)
        if rem == 0 or nfull == 0:
            nfull, tl = max(nfull, 1), False
        elif rem <= 64:
            tl = True
        else:
            nfull, tl = nfull + 1, False
        nfs.append(nfull)
        tails.append(tl)
    cfg = (*nfs, *tails, bool(np.all(pb == 0)))

    nc = _get_program(cfg)
    ins = _prep_core_inputs(query, key, value, wT, pb, keeps, cfg)
    res = run_bass_kernel_spmd(nc, ins, list(range(B)), trace=_trace,
                               **run_kwargs)
    outs = [np.asarray(res.results[b]["out"]) for b in range(B)]
    full = np.concatenate(outs, axis=0)          # (B*T, E), b-major rows
    result = full.reshape(T, B, E)
    if _trace:
        return result, res
    return result

